# revision 15
# baseline (speedup 1.0000x reference)
"""FewShotSegmentation Trainium2 kernel.

Math: for each batch b (one per NeuronCore):
  num[k, c]  = sum_{p: mask[p]==k+1} F[c, p]          (masked pooling, K=16)
  seg[p']    = argmax_k  (num[k,:] . q[:, p']) / ||num[k,:]||
The reference's den (pixel count) and query-norm cancel inside the argmax
(positive per-k / per-p' scales), and the eps clamp never binds at these
magnitudes, so neither is computed.

Layout strategy (per core):
  F, q stored (C=1024, P=4096) channel-major. Pooling contracts pixels, so
  F is transposed tile-by-tile on the PE (128x128 f32 transposes), pooled
  against one-hot mask tiles (128p, 16k), giving numT (c-major, (128,16) per
  c-chunk). Match matmuls use q tiles as stationary: out (128 p', 16 k) in
  PSUM, accumulated over 8 c-chunks, then scaled by 1/||num_k|| and
  argmax'ed on DVE via max/max_index.

Scheduling notes: walrus allows only one sync-wait on the LW part of
fp32 matmuls/transposes, so every PE instruction must collapse its deps
to <=1 semaphore: constants are built on gpsimd only, each DMA'd tile is
first touched by a throwaway PE "absorber" transpose (which carries the
DMA wait), and each chunk's PSUM->SBUF copies + epilogue all run on one
engine (alternating DVE/ACT per chunk) so matmul waits are single-engine.
"""

import os
from contextlib import ExitStack

import numpy as np

import concourse.bass as bass
import concourse.mybir as mybir
import concourse.tile as tile
from concourse import masks
from concourse.bass_utils import run_bass_kernel_spmd

B, C, H, W = 8, 1024, 64, 64
P = H * W          # 4096 pixels
K = 16             # foreground classes
PART = 128
NCH = C // PART    # 8 channel chunks
NPJ = P // PART    # 32 pixel chunks
NG = 8             # query column groups
GW = P // NG       # 512 pixels per group
JPG = GW // PART   # 4 pixel chunks per group

F32 = mybir.dt.float32
I32 = mybir.dt.int32
U32 = mybir.dt.uint32


def build_nc(debug_dots=False):
    nc = bass.Bass(target_bir_lowering=False)

    sf = nc.dram_tensor("sf", [C, P], F32, kind="ExternalInput")
    sm = nc.dram_tensor("sm", [P], I32, kind="ExternalInput")
    qf = nc.dram_tensor("qf", [C, P], F32, kind="ExternalInput")
    seg = nc.dram_tensor("seg", [P], I32, kind="ExternalOutput")
    dbg = (
        nc.dram_tensor("dbg", [P, K], F32, kind="ExternalOutput")
        if debug_dots
        else None
    )

    with ExitStack() as ctx:
        tc = ctx.enter_context(tile.TileContext(nc))
        singles = ctx.enter_context(tc.tile_pool(name="singles", bufs=1))

        identity = singles.tile([PART, PART], F32)
        masks.make_identity(nc, identity[:])

        ones_col = singles.tile([PART, 1], F32)
        nc.gpsimd.memset(ones_col[:], 1.0)
        ones_row = singles.tile([1, PART], F32)
        nc.gpsimd.memset(ones_row[:], 1.0)

        # classvec[p, k] = k+1 for every partition
        classvec_i = singles.tile([PART, K], I32)
        nc.gpsimd.iota(
            classvec_i[:], pattern=[[1, K]], base=1, channel_multiplier=0
        )
        classvec = singles.tile([PART, K], F32)
        nc.vector.tensor_copy(classvec[:], classvec_i[:])

        # mask in pixel-major layout: mask_pm[p, j] = sm[j*128 + p]
        mask_pm_i = singles.tile([PART, NPJ], I32)
        with nc.allow_non_contiguous_dma("16KB one-time strided mask load"):
            nc.sync.dma_start(
                out=mask_pm_i[:], in_=sm.rearrange("(n p) -> p n", p=PART)
            )
        mask_pm = singles.tile([PART, NPJ], F32)
        nc.vector.tensor_copy(mask_pm[:], mask_pm_i[:])

        # one-hot masks: onehot[p, j, k] = (sm[j*128+p] == k+1)
        onehot = singles.tile([PART, NPJ, K], F32)
        for j in range(NPJ):
            nc.vector.tensor_scalar(
                onehot[:, j, :],
                classvec[:],
                mask_pm[:, j : j + 1],
                None,
                op0=mybir.AluOpType.is_equal,
            )

        # pooled (transposed) prototypes numT[c, i, k], c-major
        numT = singles.tile([PART, NCH, K], F32)

        def epi_copy(sel, out, in_):
            if sel % 2 == 0:
                nc.vector.tensor_copy(out, in_)
            else:
                nc.scalar.copy(out, in_)

        # ---------------- pooling phase ----------------
        with (
            tc.tile_pool(name="fpool", bufs=2) as fpool,
            tc.tile_pool(name="spool", bufs=2) as spool,
            tc.tile_pool(name="pst", bufs=4, space=bass.MemorySpace.PSUM) as pspool,
            tc.tile_pool(name="pnum", bufs=2, space=bass.MemorySpace.PSUM) as pnpool,
            tc.tile_pool(name="pnrm", bufs=1, space=bass.MemorySpace.PSUM) as pnrm_pool,
            tc.tile_pool(name="sq", bufs=2) as sqpool,
        ):
            pnrm = pnrm_pool.tile([K, 1], F32)
            for i in range(NCH):
                F = fpool.tile([PART, P], F32)
                nc.sync.dma_start(out=F[:], in_=sf[PART * i : PART * (i + 1), :])

                S = spool.tile([PART, NPJ, PART], F32)
                for jb in range(NPJ // 4):
                    pst = pspool.tile([PART, 4, PART], F32)
                    for t in range(4):
                        j = jb * 4 + t
                        nc.tensor.transpose(
                            pst[:, t, :],
                            F[:, PART * j : PART * (j + 1)],
                            identity[:],
                        )
                    epi_copy(jb, S[:, jb * 4 : jb * 4 + 4, :], pst[:])

                pn = pnpool.tile([PART, K], F32)
                for j in range(NPJ):
                    nc.tensor.matmul(
                        pn[:],
                        lhsT=S[:, j, :],
                        rhs=onehot[:, j, :],
                        start=(j == 0),
                        stop=(j == NPJ - 1),
                    )
                epi_copy(i, numT[:, i, :], pn[:])
                # squared entries for the prototype norms
                sq = sqpool.tile([PART, K], F32)
                if i % 2 == 0:
                    nc.vector.tensor_mul(sq[:], numT[:, i, :], numT[:, i, :])
                else:
                    nc.scalar.square(sq[:], numT[:, i, :])
                nc.tensor.matmul(
                    pnrm[:],
                    lhsT=sq[:],
                    rhs=ones_col[:],
                    start=(i == 0),
                    stop=(i == NCH - 1),
                )

            # inv_norm[k] = 1/sqrt(sum_c num^2); broadcast to (128, k)
            nrm = singles.tile([K, 1], F32)
            nc.scalar.sqrt(nrm[:], pnrm[:])
        inv = singles.tile([K, 1], F32)
        nc.vector.reciprocal(inv[:], nrm[:])

        inv_bcast = singles.tile([PART, K], F32)
        with (
            tc.tile_pool(name="pinv", bufs=2, space=bass.MemorySpace.PSUM) as pinv_pool,
        ):
            pinv_row = pinv_pool.tile([1, K], F32)
            nc.tensor.transpose(pinv_row[:], inv[:], identity[:K, :K])
            inv_row = singles.tile([1, K], F32)
            nc.vector.tensor_copy(inv_row[:], pinv_row[:])
            pinv_b = pinv_pool.tile([PART, K], F32)
            nc.tensor.matmul(
                pinv_b[:], lhsT=ones_row[:], rhs=inv_row[:], start=True, stop=True
            )
            nc.vector.tensor_copy(inv_bcast[:], pinv_b[:])

        # ---------------- match phase ----------------
        outt = singles.tile([PART, NPJ], I32)
        with (
            tc.tile_pool(name="qpool", bufs=2) as qpool,
            tc.tile_pool(name="pdot", bufs=2, space=bass.MemorySpace.PSUM) as pdpool,
            tc.tile_pool(name="sc", bufs=4) as scpool,
            tc.tile_pool(name="m8", bufs=4) as m8pool,
            tc.tile_pool(name="mi", bufs=4) as mipool,
        ):
            for g in range(NG):
                Q = qpool.tile([PART, NCH, GW], F32)
                nc.sync.dma_start(
                    out=Q[:],
                    in_=qf.rearrange("(n p) q -> p n q", p=PART)[
                        :, :, GW * g : GW * (g + 1)
                    ],
                )

                pd = pdpool.tile([PART, JPG, K], F32)
                for ci in range(NCH):
                    for t in range(JPG):
                        # start=True clears has_written for the WHOLE bank,
                        # so only the very first matmul into pd may set it;
                        # later t-slices first-write via overwrite-where-clear.
                        nc.tensor.matmul(
                            pd[:, t, :],
                            lhsT=Q[:, ci, PART * t : PART * (t + 1)],
                            rhs=numT[:, ci, :],
                            start=(ci == 0 and t == 0),
                            stop=(ci == NCH - 1),
                            skip_group_check=True,
                        )
                for t in range(JPG):
                    sc = scpool.tile([PART, K], F32)
                    nc.vector.tensor_mul(sc[:], pd[:, t, :], inv_bcast[:])
                    if dbg is not None:
                        dsb = scpool.tile([PART, K], F32, tag="dbgsb")
                        nc.vector.tensor_copy(dsb[:], pd[:, t, :])
                        nc.sync.dma_start(
                            out=dbg[(g * JPG + t) * PART : (g * JPG + t + 1) * PART, :],
                            in_=dsb[:],
                        )
                    m8 = m8pool.tile([PART, 8], F32)
                    nc.vector.max(m8[:], sc[:])
                    mi = mipool.tile([PART, 8], U32)
                    nc.vector.max_index(mi[:], m8[:], sc[:])
                    j = g * JPG + t
                    nc.vector.tensor_copy(outt[:, j : j + 1], mi[:, 0:1])

        with nc.allow_non_contiguous_dma("16KB one-time strided seg store"):
            nc.sync.dma_start(
                out=seg.rearrange("(n p) -> p n", p=PART), in_=outt[:]
            )

    _hoist_excess_matmul_waits(nc)
    return nc


def _hoist_excess_matmul_waits(nc):
    """walrus allows only one sync-wait per lowered instruction for some
    instruction structs (fp32 matmul LW, pseudo-DMA, ...); hoist extras
    onto wait-only event-semaphore instructions inserted right before
    the instruction on the same queue."""
    n = 0
    for f in nc.m.functions:
        for bb in f.blocks:
            out, changed = [], False
            for ins in bb.instructions:
                if True:
                    w = list(ins.sync_info.on_wait) if ins.sync_info else []
                    if len(w) >= 2:
                        for x in w[:-1]:
                            n += 1
                            out.append(
                                mybir.InstEventSemaphore(
                                    name=f"I-wh-{n}",
                                    engine=ins.engine,
                                    ins=[],
                                    outs=[],
                                    sync_info=mybir.SyncInfo(
                                        on_wait=[x], on_update=[]
                                    ),
                                )
                            )
                        ins.sync_info = mybir.SyncInfo(
                            on_wait=[w[-1]], on_update=list(ins.sync_info.on_update)
                        )
                        changed = True
                out.append(ins)
            if changed:
                bb.instructions = out


_NC_CACHE = None


def _get_nc():
    global _NC_CACHE
    if _NC_CACHE is None:
        _NC_CACHE = build_nc()
    return _NC_CACHE


def run(inputs: dict, trace: bool = False, **kw):
    """Shard over batch, run on 8 cores, gather. Returns (seg, BassKernelResults)."""
    sf = np.ascontiguousarray(inputs["support_features"], dtype=np.float32)
    sm = np.ascontiguousarray(inputs["support_masks"], dtype=np.int32)
    qf = np.ascontiguousarray(inputs["query_features"], dtype=np.float32)
    assert sf.shape == (B, C, H, W), sf.shape
    assert sm.shape == (B, 1, H, W), sm.shape
    assert qf.shape == (B, C, H, W), qf.shape

    in_maps = [
        {
            "sf": sf[b].reshape(C, P),
            "sm": sm[b].reshape(P),
            "qf": qf[b].reshape(C, P),
        }
        for b in range(B)
    ]
    res = run_bass_kernel_spmd(
        _get_nc(), in_maps, core_ids=list(range(B)), trace=trace, **kw
    )
    seg = np.stack([res.results[b]["seg"] for b in range(B)]).reshape(B, H, W)
    return seg.astype(np.int32), res


def kernel(**inputs) -> np.ndarray:
    seg, _ = run(inputs, trace=False)
    return seg


# revision 17
# speedup vs baseline: 1.2331x; 1.2331x over previous
"""FewShotSegmentation Trainium2 kernel.

Math: for each batch b (one per NeuronCore):
  num[k, c]  = sum_{p: mask[p]==k+1} F[c, p]          (masked pooling, K=16)
  seg[p']    = argmax_k  (num[k,:] . q[:, p']) / ||num[k,:]||
The reference's den (pixel count) and query-norm cancel inside the argmax
(positive per-k / per-p' scales), and the eps clamp never binds at these
magnitudes, so neither is computed.

v2 layout strategy (per core), driven by the HW profile: fp32 matmuls cost
~4 PE-cycles per moving column plus ~2 instruction-pair overheads, so all
bulk matmuls use 512-wide moving operands and 16-wide stationaries:

  pooling:  transpose F tiles on PE (128x128), gather into S_half
            (128p, 32j, 512c); num_g' (16k, 512c) += onehot_j.T @ S_half_j
            (64 matmuls total). k-major num -> norms are free-dim reduces.
  numT:     8 small PE transposes of num (16,128) -> (128,16) c-major.
  match:    dots_g (16k, 512p') += numT_i.T @ q_i (64 matmuls), then
            ACT applies 1/||num|| as a per-partition scale, 4 small PE
            transposes per group -> (128p',16k), DVE max/max_index argmax.

Walrus in this toolchain allows only ONE sync-wait per lowered
instruction for several instruction structs; _hoist_excess_matmul_waits
post-processes the scheduled module, moving excess waits onto inserted
wait-only event-semaphore instructions.
"""

from contextlib import ExitStack

import numpy as np

import concourse.bass as bass
import concourse.mybir as mybir
import concourse.tile as tile
from concourse import masks
from concourse.bass_utils import run_bass_kernel_spmd

B, C, H, W = 8, 1024, 64, 64
P = H * W          # 4096 pixels
K = 16             # foreground classes
PART = 128
NCH = C // PART    # 8 channel chunks
NPJ = P // PART    # 32 pixel chunks
NG = 8             # query column groups
GW = P // NG       # 512 pixels per group
JPG = GW // PART   # 4 pixel chunks per group
CHW = 512          # c-columns per pooling matmul group
NCG = C // CHW     # 2 pooling column groups

F32 = mybir.dt.float32
I32 = mybir.dt.int32
U32 = mybir.dt.uint32


def build_nc():
    nc = bass.Bass(target_bir_lowering=False)

    sf = nc.dram_tensor("sf", [C, P], F32, kind="ExternalInput")
    sm = nc.dram_tensor("sm", [P], I32, kind="ExternalInput")
    qf = nc.dram_tensor("qf", [C, P], F32, kind="ExternalInput")
    seg = nc.dram_tensor("seg", [P], I32, kind="ExternalOutput")

    with ExitStack() as ctx:
        tc = ctx.enter_context(tile.TileContext(nc))
        singles = ctx.enter_context(tc.tile_pool(name="singles", bufs=1))

        identity = singles.tile([PART, PART], F32)
        masks.make_identity(nc, identity[:])

        # classvec[p, k] = k+1 for every partition
        classvec_i = singles.tile([PART, K], I32)
        nc.gpsimd.iota(classvec_i[:], pattern=[[1, K]], base=1, channel_multiplier=0)
        classvec = singles.tile([PART, K], F32)
        nc.vector.tensor_copy(classvec[:], classvec_i[:])

        # mask in pixel-major layout: mask_pm[p, j] = sm[j*128 + p]
        mask_pm_i = singles.tile([PART, NPJ], I32)
        with nc.allow_non_contiguous_dma("16KB one-time strided mask load"):
            nc.sync.dma_start(
                out=mask_pm_i[:], in_=sm.rearrange("(n p) -> p n", p=PART)
            )
        mask_pm = singles.tile([PART, NPJ], F32)
        nc.vector.tensor_copy(mask_pm[:], mask_pm_i[:])

        # one-hot masks: onehot[p, j, k] = (sm[j*128+p] == k+1)
        onehot = singles.tile([PART, NPJ, K], F32)
        for j in range(NPJ):
            nc.vector.tensor_scalar(
                onehot[:, j, :],
                classvec[:],
                mask_pm[:, j : j + 1],
                None,
                op0=mybir.AluOpType.is_equal,
            )

        # transposed support features, half of C at a time
        S_half = singles.tile([PART, NPJ, CHW], F32)
        # pooled prototypes: k-major and c-major forms
        numK = singles.tile([K, C], F32)          # (16, 1024)
        numT = singles.tile([PART, NCH, K], F32)  # c-major (128,16) per chunk
        inv = singles.tile([K, 1], F32)
        nrm2 = singles.tile([K, NCG], F32)
        outt = singles.tile([PART, NPJ], I32)

        def epi_copy(sel, out, in_):
            if sel % 2 == 0:
                nc.vector.tensor_copy(out, in_)
            else:
                nc.scalar.copy(out, in_)

        with (
            tc.tile_pool(name="fpool", bufs=2) as fpool,
            tc.tile_pool(name="qpool", bufs=2) as qpool,
            tc.tile_pool(name="scp", bufs=2) as scpool,
            tc.tile_pool(name="dtsb", bufs=4) as dtsbpool,
            tc.tile_pool(name="m8", bufs=4) as m8pool,
            tc.tile_pool(name="mi", bufs=4) as mipool,
            tc.tile_pool(name="pst", bufs=2, space=bass.MemorySpace.PSUM) as pspool,
            tc.tile_pool(name="pnum", bufs=1, space=bass.MemorySpace.PSUM) as pnpool,
            tc.tile_pool(name="dtr", bufs=2, space=bass.MemorySpace.PSUM) as dtrpool,
            tc.tile_pool(name="pdot", bufs=2, space=bass.MemorySpace.PSUM) as pdpool,
        ):
            # ---------------- pooling phase ----------------
            pnum = pnpool.tile([K, NCG, CHW], F32)  # (16, 2, 512): 2 banks
            for i in range(NCH):
                F = fpool.tile([PART, P], F32)
                nc.sync.dma_start(out=F[:], in_=sf[PART * i : PART * (i + 1), :])
                co = PART * (i % 4)  # c-offset within S_half
                for jb in range(NPJ // 4):
                    pst = pspool.tile([PART, 4, PART], F32)
                    for t in range(4):
                        j = jb * 4 + t
                        nc.tensor.transpose(
                            pst[:, t, :],
                            F[:, PART * j : PART * (j + 1)],
                            identity[:],
                        )
                    epi_copy(
                        jb, S_half[:, jb * 4 : jb * 4 + 4, co : co + PART], pst[:]
                    )

                if i % 4 == 3:
                    gp = i // 4  # pooling column group (0 or 1)
                    for j in range(NPJ):
                        nc.tensor.matmul(
                            pnum[:, gp, :],
                            lhsT=onehot[:, j, :],
                            rhs=S_half[:, j, :],
                            start=(j == 0),
                            stop=(j == NPJ - 1),
                            skip_group_check=True,
                        )
                    # k-major num for this half of C
                    nc.scalar.copy(
                        numK[:, CHW * gp : CHW * (gp + 1)], pnum[:, gp, :]
                    )

            # norms (free-dim reduce in k-major layout) and 1/||num||
            sqs = scpool.tile([K, CHW], F32, tag="sq")
            nc.scalar.square(sqs[:], numK[:, 0:CHW])
            nc.vector.reduce_sum(nrm2[:, 0:1], sqs[:], axis=mybir.AxisListType.X)
            sqs2 = scpool.tile([K, CHW], F32, tag="sq")
            nc.scalar.square(sqs2[:], numK[:, CHW : 2 * CHW])
            nc.vector.reduce_sum(nrm2[:, 1:2], sqs2[:], axis=mybir.AxisListType.X)
            nrm = singles.tile([K, 1], F32)
            nc.vector.tensor_add(nrm[:], nrm2[:, 0:1], nrm2[:, 1:2])
            nc.scalar.sqrt(nrm[:], nrm[:])
            nc.vector.reciprocal(inv[:], nrm[:])

            # c-major numT via small PE transposes
            for i in range(NCH):
                dtr = dtrpool.tile([PART, K], F32)
                nc.tensor.transpose(
                    dtr[:],
                    numK[:, PART * i : PART * (i + 1)],
                    identity[:K, :K],
                )
                epi_copy(i, numT[:, i, :], dtr[:])

            # ---------------- match phase ----------------
            for g in range(NG):
                Q = qpool.tile([PART, NCH, GW], F32)
                nc.sync.dma_start(
                    out=Q[:],
                    in_=qf.rearrange("(n p) q -> p n q", p=PART)[
                        :, :, GW * g : GW * (g + 1)
                    ],
                )
                pd = pdpool.tile([K, GW], F32)
                for i in range(NCH):
                    nc.tensor.matmul(
                        pd[:],
                        lhsT=numT[:, i, :],
                        rhs=Q[:, i, :],
                        start=(i == 0),
                        stop=(i == NCH - 1),
                    )
                # scale by 1/||num|| (per-partition) while leaving PSUM
                sck = scpool.tile([K, GW], F32, tag="sck")
                nc.scalar.mul(sck[:], pd[:], inv[:])
                for t in range(JPG):
                    dtr = dtrpool.tile([PART, K], F32)
                    nc.tensor.transpose(
                        dtr[:],
                        sck[:, PART * t : PART * (t + 1)],
                        identity[:K, :K],
                    )
                    dt = dtsbpool.tile([PART, K], F32)
                    nc.vector.tensor_copy(dt[:], dtr[:])
                    m8 = m8pool.tile([PART, 8], F32)
                    nc.vector.max(m8[:], dt[:])
                    mi = mipool.tile([PART, 8], U32)
                    nc.vector.max_index(mi[:], m8[:], dt[:])
                    j = g * JPG + t
                    nc.vector.tensor_copy(outt[:, j : j + 1], mi[:, 0:1])

            with nc.allow_non_contiguous_dma("16KB one-time strided seg store"):
                nc.sync.dma_start(
                    out=seg.rearrange("(n p) -> p n", p=PART), in_=outt[:]
                )

    _hoist_excess_matmul_waits(nc)
    return nc


def _hoist_excess_matmul_waits(nc):
    """walrus allows only one sync-wait per lowered instruction for some
    instruction structs (fp32 matmul LW, pseudo-DMA, ...); hoist extras
    onto wait-only event-semaphore instructions inserted right before
    the instruction on the same queue."""
    n = 0
    for f in nc.m.functions:
        for bb in f.blocks:
            out, changed = [], False
            for ins in bb.instructions:
                w = list(ins.sync_info.on_wait) if ins.sync_info else []
                if len(w) >= 2:
                    for x in w[:-1]:
                        n += 1
                        out.append(
                            mybir.InstEventSemaphore(
                                name=f"I-wh-{n}",
                                engine=ins.engine,
                                ins=[],
                                outs=[],
                                sync_info=mybir.SyncInfo(on_wait=[x], on_update=[]),
                            )
                        )
                    ins.sync_info = mybir.SyncInfo(
                        on_wait=[w[-1]], on_update=list(ins.sync_info.on_update)
                    )
                    changed = True
                out.append(ins)
            if changed:
                bb.instructions = out


_NC_CACHE = None


def _get_nc():
    global _NC_CACHE
    if _NC_CACHE is None:
        _NC_CACHE = build_nc()
    return _NC_CACHE


def run(inputs: dict, trace: bool = False, **kw):
    """Shard over batch, run on 8 cores, gather. Returns (seg, BassKernelResults)."""
    sf = np.ascontiguousarray(inputs["support_features"], dtype=np.float32)
    sm = np.ascontiguousarray(inputs["support_masks"], dtype=np.int32)
    qf = np.ascontiguousarray(inputs["query_features"], dtype=np.float32)
    assert sf.shape == (B, C, H, W), sf.shape
    assert sm.shape == (B, 1, H, W), sm.shape
    assert qf.shape == (B, C, H, W), qf.shape

    in_maps = [
        {
            "sf": sf[b].reshape(C, P),
            "sm": sm[b].reshape(P),
            "qf": qf[b].reshape(C, P),
        }
        for b in range(B)
    ]
    res = run_bass_kernel_spmd(
        _get_nc(), in_maps, core_ids=list(range(B)), trace=trace, **kw
    )
    seg = np.stack([res.results[b]["seg"] for b in range(B)]).reshape(B, H, W)
    return seg.astype(np.int32), res


def kernel(**inputs) -> np.ndarray:
    seg, _ = run(inputs, trace=False)
    return seg
